# revision 1
# baseline (speedup 1.0000x reference)
"""BitLinear forward kernel for Trainium2 (8 NeuronCores, data-parallel).

Computes y = sign(x) @ (alpha * code)^T + b where code/alpha are the
per-row ternarization of W (BitNet-style, delta_w = 0.05, delta_a = 0.0).

Sharding: x is split over batch*seq (16384 rows) across 8 cores; W is
replicated (each core quantizes the full W on-device); outputs are
concatenated on the host.

The matmul runs in fp8 DoubleRow mode (operand values are exactly
{-1, 0, +1}; two d-tiles are paired per PE pass) with fp32 PSUM
accumulation, so integer counts are exact; the per-output-feature alpha
scale is applied in fp32 on eviction. b from setup_inputs() is zeros; a
nonzero b takes a second elementwise pass.

Layout: the contraction dim (d) must live on SBUF partitions for both
matmul operands, so sign(x) and code are bounced through DRAM (bf16) in
half-row blocks and read back through the DMA xbar transpose, then cast
(one wide DVE op each) into paired-d-tile fp8 operands [128, 2, 1024].
Half-width PSUM groups decouple the low o-banks from the second half of
the W pipeline so the PE can start early.

alpha identity: sum(aWc * (aWc>=thr)) == sum(relu(aWc-thr)) + thr*count,
so the numerator comes from an ACT Relu pass with accumulate.
"""

import sys

for _p in ("/opt/trn_rl_repo", "/opt/trn_rl_repo/concourse"):
    if _p not in sys.path:
        sys.path.insert(0, _p)

import numpy as np

import concourse.bass as bass
import concourse.tile as tile
import concourse.mybir as mybir
from concourse import bacc
from concourse.bass_utils import run_bass_kernel_spmd

# Problem shape (hardcoded per contract)
B, S, D, O = 4, 4096, 2048, 2048
N_CORES = 8
T = (B * S) // N_CORES  # 2048 token rows per core
DELTA_W = 0.05

P = 128
TT = T // P   # 16 t-tiles
DT = D // P   # 16 d-tiles
DP = DT // 2  # 8 d-pair tiles (DoubleRow)
WT = O // P   # 16 W row-tiles
NB = 4        # psum banks per t-tile (512 f32 each)
NBW = O // NB  # 512
H = 2         # half blocks
HR = T // H   # 1024 rows per half

F32 = mybir.dt.float32
BF16 = mybir.dt.bfloat16
FP8 = mybir.dt.float8e4
U16 = mybir.dt.uint16

_CACHE = {}


def _build(with_bias: bool):
    nc = bacc.Bacc("TRN2", target_bir_lowering=False, debug=False,
                   num_devices=N_CORES)
    x_d = nc.dram_tensor("x", [T, D], F32, kind="ExternalInput").ap()
    w_d = nc.dram_tensor("W", [O, D], F32, kind="ExternalInput").ap()
    y_d = nc.dram_tensor("y", [T, O], F32, kind="ExternalOutput").ap()
    if with_bias:
        b_d = nc.dram_tensor("b", [O], F32, kind="ExternalInput").ap()

    with tile.TileContext(nc) as tc:
        with (
            tc.tile_pool(name="dram", bufs=8, space="DRAM") as dram,
            tc.tile_pool(name="wload", bufs=3) as wload,
            tc.tile_pool(name="awc", bufs=2) as awc_pool,
            tc.tile_pool(name="junk", bufs=1) as junk_pool,
            tc.tile_pool(name="wsmall", bufs=2) as wsmall,
            tc.tile_pool(name="stats", bufs=1) as stats,
            tc.tile_pool(name="xload", bufs=2) as xload,
            tc.tile_pool(name="xsign", bufs=2) as xsign,
            tc.tile_pool(name="tpose", bufs=4) as tpose,
            tc.tile_pool(name="codeT", bufs=DP * H) as codeT_pool,
            tc.tile_pool(name="xqT", bufs=DP * H) as xqT_pool,
            tc.tile_pool(name="psum", bufs=4, space="PSUM") as psum_pool,
            tc.tile_pool(name="yout", bufs=3) as yout,
            tc.tile_pool(name="bcast", bufs=1) as bcast,
        ):
            # half-row DRAM bounce tiles
            xq_dram = [dram.tile([HR, D], BF16, tag=f"xqd{h}",
                                 name=f"xq_dram{h}") for h in range(H)]
            code_dram = [dram.tile([HR, D], BF16, tag=f"cdd{h}",
                                   name=f"code_dram{h}") for h in range(H)]
            alpha_dram = dram.tile([O], F32, tag="alphad")

            # Per-row stats, one column per W row-tile
            S_all = stats.tile([P, WT], F32, tag="S")
            T_all = stats.tile([P, WT], F32, tag="T")
            den_all = stats.tile([P, WT], F32, tag="den")
            relu_all = stats.tile([P, WT], F32, tag="relu")
            negmean_all = stats.tile([P, WT], F32, tag="negmean")
            thr_all = stats.tile([P, WT], F32, tag="thr")
            negthr_all = stats.tile([P, WT], F32, tag="negthr")
            alpha_all = stats.tile([P, WT], F32, tag="alpha")

            act_junk = junk_pool.tile([P, D], F32, tag="act_junk")
            alphaB = bcast.tile([P, O], F32, tag="alphaB")

            # big pair tiles [128, 2*1024] fp8:
            #   codeT8[dp][h]: halves = d-tiles (2dp, 2dp+1); free covers
            #   o in [h*1024, (h+1)*1024) (banks 2h, 2h+1)
            #   xqT8[dp][h]: free covers t in [h*1024, (h+1)*1024)
            codeT8 = [[codeT_pool.tile([P, 2 * HR], FP8, tag="codeT",
                                       name=f"codeT{dp}_{h}")
                       for h in range(H)] for dp in range(DP)]
            xqT8 = [[xqT_pool.tile([P, 2 * HR], FP8, tag="xqT",
                                   name=f"xqT{dp}_{h}")
                     for h in range(H)] for dp in range(DP)]

            for h in range(H):
                # ---- half h of the W pipeline (wi = 8h..8h+7) -----------
                for j in range(WT // H):
                    wi = h * (WT // H) + j
                    wt = wload.tile([P, D], F32)
                    nc.gpsimd.dma_start(wt[:], w_d[wi * P:(wi + 1) * P, :])
                    # S = sum(W) via ACT Copy with accumulate
                    nc.scalar.activation(
                        out=act_junk[:], in_=wt[:],
                        func=mybir.ActivationFunctionType.Copy,
                        accum_out=S_all[:, wi:wi + 1],
                    )
                    nc.scalar.mul(
                        negmean_all[:, wi:wi + 1], S_all[:, wi:wi + 1],
                        -1.0 / D,
                    )
                    # aWc = |W - mean|, T = sum(aWc)
                    aWc = awc_pool.tile([P, D], F32)
                    nc.scalar.activation(
                        out=aWc[:], in_=wt[:],
                        func=mybir.ActivationFunctionType.Abs,
                        bias=negmean_all[:, wi:wi + 1],
                        accum_out=T_all[:, wi:wi + 1],
                    )
                    nc.scalar.mul(
                        thr_all[:, wi:wi + 1], T_all[:, wi:wi + 1],
                        DELTA_W / D,
                    )
                    nc.scalar.mul(
                        negthr_all[:, wi:wi + 1], T_all[:, wi:wi + 1],
                        -DELTA_W / D,
                    )
                    # sgn = Sign(W - mean) bf16
                    sgn = wsmall.tile([P, D], BF16, tag="sgn")
                    nc.scalar.activation(
                        out=sgn[:], in_=wt[:],
                        func=mybir.ActivationFunctionType.Sign,
                        bias=negmean_all[:, wi:wi + 1],
                    )
                    # R = sum(relu(aWc - thr))  (alpha numerator part)
                    nc.scalar.activation(
                        out=act_junk[:], in_=aWc[:],
                        func=mybir.ActivationFunctionType.Relu,
                        bias=negthr_all[:, wi:wi + 1],
                        accum_out=relu_all[:, wi:wi + 1],
                    )
                    # s01 = (aWc >= thr), den = count
                    s01 = wsmall.tile([P, D], BF16, tag="s01")
                    nc.vector.tensor_scalar(
                        out=s01[:], in0=aWc[:],
                        scalar1=thr_all[:, wi:wi + 1], scalar2=0.0,
                        op0=mybir.AluOpType.is_ge,
                        op1=mybir.AluOpType.add,
                        accum_out=den_all[:, wi:wi + 1],
                    )
                    # code = sgn * s01 (values exactly -1/0/1)
                    code = wsmall.tile([P, D], BF16, tag="code")
                    nc.vector.tensor_mul(code[:], sgn[:], s01[:])
                    nc.gpsimd.dma_start(
                        code_dram[h][j * P:(j + 1) * P, :], code[:])

                    # ---- same-index t-tile of the x pipeline ------------
                    ti = h * (TT // H) + j
                    xb = xload.tile([P, D], BF16)
                    nc.gpsimd.dma_start(
                        xb[:], x_d[ti * P:(ti + 1) * P, :])  # f32->bf16
                    xq = xsign.tile([P, D], BF16)
                    nc.vector.tensor_scalar(
                        out=xq.bitcast(U16)[:], in0=xb.bitcast(U16)[:],
                        scalar1=0x8000, scalar2=0x3F80,
                        op0=mybir.AluOpType.bitwise_and,
                        op1=mybir.AluOpType.bitwise_or,
                    )
                    nc.gpsimd.dma_start(
                        xq_dram[h][j * P:(j + 1) * P, :], xq[:])

                # ---- half h transposed reads + fp8 pair converts --------
                for di in range(DT):
                    dp, hd = divmod(di, 2)
                    tb = tpose.tile([P, HR], BF16, tag="tp_code")
                    nc.sync.dma_start_transpose(
                        tb[:], code_dram[h][:, di * P:(di + 1) * P])
                    nc.vector.tensor_copy(
                        out=codeT8[dp][h][:, hd * HR:(hd + 1) * HR],
                        in_=tb[:])
                    tb2 = tpose.tile([P, HR], BF16, tag="tp_xq")
                    nc.sync.dma_start_transpose(
                        tb2[:], xq_dram[h][:, di * P:(di + 1) * P])
                    nc.vector.tensor_copy(
                        out=xqT8[dp][h][:, hd * HR:(hd + 1) * HR],
                        in_=tb2[:])

                # ---- per-half alpha = (R + thr*den)/max(den,1) ----------
                WH = WT // H
                hs = slice(h * WH, (h + 1) * WH)
                num = stats.tile([P, WT], F32, tag="num")
                nc.vector.tensor_mul(
                    num[:, hs], thr_all[:, hs], den_all[:, hs])
                nc.vector.tensor_add(num[:, hs], num[:, hs], relu_all[:, hs])
                denc = stats.tile([P, WT], F32, tag="denc")
                nc.vector.tensor_scalar_max(denc[:, hs], den_all[:, hs], 1.0)
                rden = stats.tile([P, WT], F32, tag="rden")
                nc.vector.reciprocal(rden[:, hs], denc[:, hs])
                nc.vector.tensor_mul(
                    alpha_all[:, hs], num[:, hs], rden[:, hs])
                nc.sync.dma_start(
                    alpha_dram[h * (O // H):(h + 1) * (O // H)].rearrange(
                        "(w p) -> p w", p=P)[:, :],
                    alpha_all[:, hs])
                nc.gpsimd.dma_start(
                    alphaB[:, h * (O // H):(h + 1) * (O // H)],
                    alpha_dram[h * (O // H):(h + 1) * (O // H)].unsqueeze(
                        0).to_broadcast((P, O // H)))

            if with_bias:
                biasB = bcast.tile([P, O], F32, tag="biasB")
                nc.gpsimd.dma_start(
                    biasB[:], b_d.unsqueeze(0).to_broadcast((P, O)))

            # -------- main matmul (DoubleRow fp8, half-width groups) -----
            def mm_group(ti, oh):
                """Accumulate y[ti-block, oh*1024:(oh+1)*1024]."""
                ps = psum_pool.tile([P, O // H], F32, tag="ps", name=f"ps{ti}_{oh}")
                q, r = divmod(ti, TT // H)
                for dp in range(DP):
                    lhsT = xqT8[dp][q].rearrange(
                        "p (two m) -> p two m", two=2)[:, :, r * P:(r + 1) * P]
                    rhs_all = codeT8[dp][oh].rearrange(
                        "p (two n) -> p two n", two=2)
                    for bk in range(NB // H):
                        nc.tensor.matmul(
                            ps[:, bk * NBW:(bk + 1) * NBW],
                            lhsT,
                            rhs_all[:, :, bk * NBW:(bk + 1) * NBW],
                            start=(dp == 0),
                            stop=(dp == DP - 1),
                            perf_mode=mybir.MatmulPerfMode.DoubleRow,
                        )
                ysb = yout.tile([P, O // H], F32, tag="ysb")
                nc.vector.tensor_mul(
                    ysb[:], ps[:], alphaB[:, oh * (O // H):(oh + 1) * (O // H)])
                if with_bias:
                    nc.vector.tensor_add(
                        ysb[:], ysb[:],
                        biasB[:, oh * (O // H):(oh + 1) * (O // H)])
                nc.gpsimd.dma_start(
                    y_d[ti * P:(ti + 1) * P,
                        oh * (O // H):(oh + 1) * (O // H)], ysb[:])

            # low o-half for all t-tiles first (only needs W-half 0),
            # then the high o-half.
            for oh in range(H):
                for ti in range(TT):
                    mm_group(ti, oh)

    nc.compile()
    return nc


def _get_nc(with_bias: bool):
    key = with_bias
    if key not in _CACHE:
        _CACHE[key] = _build(with_bias)
    return _CACHE[key]


def kernel(x: np.ndarray, W: np.ndarray, b: np.ndarray) -> np.ndarray:
    x = np.asarray(x, dtype=np.float32)
    W = np.ascontiguousarray(W, dtype=np.float32)
    b = np.asarray(b, dtype=np.float32)
    with_bias = bool(np.any(b))

    nc = _get_nc(with_bias)

    xf = np.ascontiguousarray(x.reshape(B * S, D))
    in_maps = []
    for c in range(N_CORES):
        m = {"x": np.ascontiguousarray(xf[c * T:(c + 1) * T]), "W": W}
        if with_bias:
            m["b"] = b
        in_maps.append(m)

    res = run_bass_kernel_spmd(nc, in_maps, core_ids=list(range(N_CORES)))
    y = np.concatenate([res.results[c]["y"] for c in range(N_CORES)], axis=0)
    return np.ascontiguousarray(y.reshape(B, S, O))


if __name__ == "__main__":
    rng = np.random.default_rng(0)
    x = rng.standard_normal((B, S, D), dtype=np.float32)
    W = rng.standard_normal((O, D), dtype=np.float32) * 0.03
    b = np.zeros((O,), dtype=np.float32)
    y = kernel(x, W, b)
    print("kernel ran, y shape", y.shape, "mean|y|", np.abs(y).mean())



# revision 11
# speedup vs baseline: 3.4114x; 3.4114x over previous
"""BitLinear forward kernel for Trainium2 (8 NeuronCores).

y = sign(x) @ (alpha * code)^T + b, with code/alpha the per-row
ternarization of W (BitNet, delta_w = 0.05, delta_a = 0).

Strategy vs the DMA-transpose baseline (474 us):
  * x is staged to DRAM pre-transposed in bf16 (layout choice made while
    sharding on the host), so the matmul lhs needs no on-device
    transpose and x HBM read traffic is halved.
  * The output is computed transposed (yT [o, t], bf16) so the per-row
    alpha becomes a per-partition scale applied during PSUM eviction;
    the host transposes back.
  * W quantization is sharded: core c quantizes rows [256c, 256c+256),
    PE-transposes its code block, and the fp8 codeT blocks + alphas are
    AllGather'd across the 8 cores.  This cuts the elementwise
    quantization work and the W read traffic by 8x.
  * code is computed as Sign(Wc - thr) + Sign(Wc + thr) in {-2, 0, 2}
    (two ACT passes, with accum_out giving den and alpha for free); the
    factor 2 is folded into the eviction scale.
  * Matmul runs fp8 DoubleRow (0.5 cycles/row) with f32 PSUM, exact for
    these integer-valued operands.
"""

import sys

for _p in ("/opt/trn_rl_repo", "/opt/trn_rl_repo/concourse"):
    if _p not in sys.path:
        sys.path.insert(0, _p)

import numpy as np

import concourse.bass as bass
import concourse.tile as tile
import concourse.mybir as mybir
from concourse import bacc
from concourse.bass_utils import run_bass_kernel_spmd

B, S, D, O = 4, 4096, 2048, 2048
N_CORES = 8
T = (B * S) // N_CORES      # 2048 token rows per core
DELTA_W = 0.05
P = 128
DP = D // 256               # 8 paired-d slabs (DoubleRow)
OT = O // P                 # 16 output row tiles
WPC = O // N_CORES          # 256 W rows quantized per core
MWT = WPC // P              # 2 local W row-tiles

F32 = mybir.dt.float32
BF16 = mybir.dt.bfloat16
FP8 = mybir.dt.float8e4

NP_BF16 = mybir.dt.np(BF16)

Alu = mybir.AluOpType
Act = mybir.ActivationFunctionType

_CACHE = {}


def _build(with_bias: bool):
    nc = bacc.Bacc("TRN2", target_bir_lowering=False, debug=False,
                   num_devices=N_CORES)
    xT_d = nc.dram_tensor("xT", [D, T], BF16, kind="ExternalInput").ap()
    w_d = nc.dram_tensor("Wc", [WPC, D], F32, kind="ExternalInput").ap()
    eye_d = nc.dram_tensor("eye", [P, P], F32, kind="ExternalInput").ap()
    yT_d = nc.dram_tensor("yT", [O, T], BF16, kind="ExternalOutput").ap()
    if with_bias:
        b_d = nc.dram_tensor("b", [O], F32, kind="ExternalInput").ap()

    groups = [list(range(N_CORES))]

    with tile.TileContext(nc) as tc:
        with (
            tc.tile_pool(name="dram", bufs=4, space="DRAM") as dram,
            tc.tile_pool(name="wload", bufs=2) as wload,
            tc.tile_pool(name="junk", bufs=1) as junk_pool,
            tc.tile_pool(name="gp", bufs=2) as gpool,
            tc.tile_pool(name="code2", bufs=2) as code2_pool,
            tc.tile_pool(name="stats", bufs=1) as stats,
            tc.tile_pool(name="stg", bufs=1) as stg_pool,
            tc.tile_pool(name="xstage", bufs=2) as xstage_pool,
            tc.tile_pool(name="xqT", bufs=DP) as xqT_pool,
            tc.tile_pool(name="codeT", bufs=N_CORES) as codeT_pool,
            tc.tile_pool(name="small", bufs=1) as small,
            tc.tile_pool(name="ysb", bufs=4) as ysb_pool,
            tc.tile_pool(name="tp_ps", bufs=1, space="PSUM") as tp_ps_pool,
            tc.tile_pool(name="mm_ps", bufs=2, space="PSUM") as mm_ps_pool,
            tc.tile_pool(name="a_ps", bufs=1, space="PSUM") as a_ps_pool,
        ):
            # ---- DRAM bounce buffers for the collectives ----------------
            ccin = dram.tile([D, 256], FP8, tag="ccin",
                             name="ccin")  # [2048 d, 256 o] fp8
            ccout = dram.tile([N_CORES * D, 256], FP8, tag="ccout",
                              name="ccout", addr_space="Shared")
            ccin_a = dram.tile([WPC], F32, tag="ccina", name="ccin_a")
            ccout_a = dram.tile([O], F32, tag="ccouta", name="ccout_a",
                                addr_space="Shared")

            # ---- identity tiles for PE transposes -----------------------
            eye_bf = small.tile([P, P], BF16, tag="eyebf")
            nc.gpsimd.dma_start(eye_bf[:], eye_d[:, :])
            eye16 = small.tile([16, 16], F32, tag="eye16")
            nc.gpsimd.dma_start(eye16[:], eye_d[0:16, 0:16])

            # ---- per-row stats ([128, MWT]) -----------------------------
            S_t = stats.tile([P, MWT], F32, tag="S")
            negmean = stats.tile([P, MWT], F32, tag="negmean")
            T_t = stats.tile([P, MWT], F32, tag="T")
            thr = stats.tile([P, MWT], F32, tag="thr")
            bp = stats.tile([P, MWT], F32, tag="bp")
            bm = stats.tile([P, MWT], F32, tag="bm")
            Sg1 = stats.tile([P, MWT], F32, tag="Sg1")
            Sg2 = stats.tile([P, MWT], F32, tag="Sg2")
            A1 = stats.tile([P, MWT], F32, tag="A1")
            A2 = stats.tile([P, MWT], F32, tag="A2")
            den = stats.tile([P, MWT], F32, tag="den")
            t1 = stats.tile([P, MWT], F32, tag="t1")
            num = stats.tile([P, MWT], F32, tag="num")
            rden = stats.tile([P, MWT], F32, tag="rden")
            scale_a = stats.tile([P, MWT], F32, tag="scalea")

            junk_f = junk_pool.tile([P, D], F32, tag="junkf")
            junk_b = junk_pool.tile([P, D], BF16, tag="junkb")

            # fp8 staging of this core's codeT block [128, dt*256 + wi*128 + o]
            stg = stg_pool.tile([P, 16 * 256], FP8, tag="stg")

            # ---- W quantization (own rows only) -------------------------
            for wi in range(MWT):
                wt = wload.tile([P, D], F32, tag="wt", name=f"wt_{wi}")
                nc.gpsimd.dma_start(wt[:], w_d[wi * P:(wi + 1) * P, :])
                ws = slice(wi, wi + 1)
                # S = sum(W) ; mean
                nc.vector.tensor_scalar(
                    out=junk_f[:], in0=wt[:], scalar1=0.0, scalar2=0.0,
                    op0=Alu.add, op1=Alu.add, accum_out=S_t[:, ws])
                nc.vector.tensor_scalar_mul(negmean[:, ws], S_t[:, ws],
                                            -1.0 / D)
                # T = sum |W - mean|
                nc.scalar.activation(
                    out=junk_b[:], in_=wt[:], func=Act.Abs,
                    bias=negmean[:, ws], accum_out=T_t[:, ws])
                nc.vector.tensor_scalar_mul(thr[:, ws], T_t[:, ws],
                                            DELTA_W / D)
                # bp = -(mean + thr), bm = -(mean - thr)
                nc.vector.tensor_sub(bp[:, ws], negmean[:, ws], thr[:, ws])
                nc.vector.tensor_add(bm[:, ws], negmean[:, ws], thr[:, ws])
                # g1 = Sign(W - mean - thr), g2 = Sign(W - mean + thr)
                g1 = gpool.tile([P, D], BF16, tag="g1", name=f"g1_{wi}")
                g2 = gpool.tile([P, D], BF16, tag="g2", name=f"g2_{wi}")
                nc.scalar.activation(out=g1[:], in_=wt[:], func=Act.Sign,
                                     bias=bp[:, ws], accum_out=Sg1[:, ws])
                nc.scalar.activation(out=g2[:], in_=wt[:], func=Act.Sign,
                                     bias=bm[:, ws], accum_out=Sg2[:, ws])
                # code2 = g1 + g2 in {-2, 0, 2};  A2 = sum(code2)
                code2 = code2_pool.tile([P, D], BF16, tag="code2", name=f"code2_{wi}")
                nc.vector.scalar_tensor_tensor(
                    out=code2[:], in0=g1[:], scalar=0.0, in1=g2[:],
                    op0=Alu.add, op1=Alu.add, accum_out=A2[:, ws])
                # A1 = sum(W * code2)
                nc.vector.scalar_tensor_tensor(
                    out=junk_b[:], in0=wt[:], scalar=0.0, in1=code2[:],
                    op0=Alu.add, op1=Alu.mult, accum_out=A1[:, ws])
                # transpose code2 -> psum (bf16), evict to fp8 staging
                tp = tp_ps_pool.tile([P, D], BF16, tag="tp", name=f"tp_{wi}")
                for dt in range(16):
                    nc.tensor.transpose(
                        tp[:, dt * P:(dt + 1) * P],
                        code2[:, dt * P:(dt + 1) * P], eye_bf[:])
                nc.vector.tensor_copy(
                    out=stg[:].rearrange("p (dt o) -> p dt o", dt=16)
                        [:, :, wi * P:(wi + 1) * P],
                    in_=tp[:].rearrange("p (dt o) -> p dt o", dt=16))

            # den = D + (Sg1 - Sg2)/2
            nc.vector.tensor_sub(t1[:], Sg1[:], Sg2[:])
            nc.vector.tensor_scalar(
                out=den[:], in0=t1[:], scalar1=0.5, scalar2=float(D),
                op0=Alu.mult, op1=Alu.add)
            nc.vector.tensor_scalar_max(den[:], den[:], 1.0)
            # eviction scale = alpha/2 = (A1 + negmean*A2) / (4*den)
            nc.vector.tensor_mul(num[:], negmean[:], A2[:])
            nc.vector.tensor_add(num[:], num[:], A1[:])
            nc.vector.reciprocal(rden[:], den[:])
            nc.vector.tensor_mul(scale_a[:], num[:], rden[:])
            nc.vector.tensor_scalar_mul(scale_a[:], scale_a[:], 0.25)

            # ---- ship codeT + alpha through AllGathers ------------------
            nc.gpsimd.dma_start(
                ccin[:].rearrange("(dt p) o -> p dt o", p=P),
                stg[:].rearrange("p (dt o) -> p dt o", dt=16))
            nc.gpsimd.collective_compute(
                "AllGather", Alu.bypass, replica_groups=groups,
                ins=[ccin[:].opt()], outs=[ccout[:].opt()])
            nc.gpsimd.dma_start(
                ccin_a[:].rearrange("(w p) -> p w", p=P), scale_a[:])
            nc.gpsimd.collective_compute(
                "AllGather", Alu.bypass, replica_groups=groups,
                ins=[ccin_a[:].opt()], outs=[ccout_a[:].opt()])

            # ---- x: load transposed slabs, sign into fp8 ----------------
            xqT = []
            for dp in range(DP):
                xs = xstage_pool.tile([P, 2 * T], BF16, tag="xs", name=f"xs_{dp}")
                nc.sync.dma_start(
                    xs[:].rearrange("p (k t) -> p k t", k=2),
                    xT_d[dp * 256:(dp + 1) * 256, :].rearrange(
                        "(k p) t -> p k t", p=P))
                xq = xqT_pool.tile([P, 2 * T], FP8, tag="xqT",
                                   name=f"xqT_{dp}")
                nc.scalar.activation(out=xq[:], in_=xs[:], func=Act.Sign)
                xqT.append(xq)

            # ---- pull gathered codeT into SBUF --------------------------
            codeT = []
            for cc in range(N_CORES):
                ct = codeT_pool.tile([P, 16 * 256], FP8, tag="codeT",
                                     name=f"codeT_{cc}")
                nc.gpsimd.dma_start(
                    ct[:].rearrange("p (dt o) -> p dt o", dt=16),
                    ccout[cc * D:(cc + 1) * D, :].rearrange(
                        "(dt p) o -> p dt o", p=P))
                codeT.append(ct)

            # ---- gathered alpha -> per-partition scale columns ----------
            a16 = small.tile([16, P], F32, tag="a16")
            nc.sync.dma_start(a16[:],
                              ccout_a[:].rearrange("(j p) -> j p", j=16))
            a_ps = a_ps_pool.tile([P, 16], F32, tag="aps")
            nc.tensor.transpose(a_ps[:], a16[:], eye16[:])
            alpha_sb = small.tile([P, 16], F32, tag="alphasb")
            nc.vector.tensor_copy(out=alpha_sb[:], in_=a_ps[:])
            if with_bias:
                b16 = small.tile([16, P], F32, tag="b16")
                nc.sync.dma_start(b16[:],
                                  b_d[:].rearrange("(j p) -> j p", j=16))
                b_ps = a_ps_pool.tile([P, 16], F32, tag="bps")
                nc.tensor.transpose(b_ps[:], b16[:], eye16[:])
                bias_sb = small.tile([P, 16], F32, tag="biassb")
                nc.vector.tensor_copy(out=bias_sb[:], in_=b_ps[:])

            # ---- main matmul: yT[o, t] = codeT^T @ xqT ------------------
            for j in range(OT):
                cc, oo = divmod(j, 2)
                ctv = codeT[cc][:].rearrange("p (dt o) -> p dt o", dt=16)
                lhsT_j = ctv[:, :, oo * P:(oo + 1) * P]
                for hh in range(2):
                    ps = mm_ps_pool.tile([P, T // 2], F32, tag="ps", name=f"ps{j}_{hh}")
                    for dp in range(DP):
                        lhsT = lhsT_j[:, 2 * dp:2 * dp + 2, :]
                        for bk in range(2):
                            t0 = hh * (T // 2) + bk * 512
                            rhs = xqT[dp][:].rearrange(
                                "p (k t) -> p k t", k=2)[:, :, t0:t0 + 512]
                            nc.tensor.matmul(
                                ps[:, bk * 512:(bk + 1) * 512], lhsT, rhs,
                                start=(dp == 0), stop=(dp == DP - 1),
                                perf_mode=mybir.MatmulPerfMode.DoubleRow)
                    ysb = ysb_pool.tile([P, T // 2], BF16, tag="ysb", name=f"ysb{j}_{hh}")
                    if (j + hh) % 2 == 0:
                        nc.scalar.activation(
                            out=ysb[:], in_=ps[:], func=Act.Copy,
                            scale=alpha_sb[:, j:j + 1],
                            bias=(bias_sb[:, j:j + 1] if with_bias else 0.0))
                    else:
                        if with_bias:
                            nc.vector.tensor_scalar(
                                out=ysb[:], in0=ps[:],
                                scalar1=alpha_sb[:, j:j + 1],
                                scalar2=bias_sb[:, j:j + 1],
                                op0=Alu.mult, op1=Alu.add)
                        else:
                            nc.vector.tensor_scalar_mul(
                                ysb[:], ps[:], alpha_sb[:, j:j + 1])
                    nc.scalar.dma_start(
                        yT_d[j * P:(j + 1) * P,
                             hh * (T // 2):(hh + 1) * (T // 2)], ysb[:])

    nc.compile()
    return nc


def _get_nc(with_bias: bool):
    key = with_bias
    if key not in _CACHE:
        _CACHE[key] = _build(with_bias)
    return _CACHE[key]


def _build_in_maps(x: np.ndarray, W: np.ndarray, b: np.ndarray,
                   with_bias: bool):
    xf = x.reshape(B * S, D)
    eye = np.eye(P, dtype=np.float32)
    in_maps = []
    for c in range(N_CORES):
        m = {
            "xT": np.ascontiguousarray(xf[c * T:(c + 1) * T].T).astype(
                NP_BF16),
            "Wc": np.ascontiguousarray(W[c * WPC:(c + 1) * WPC]),
            "eye": eye,
        }
        if with_bias:
            m["b"] = b
        in_maps.append(m)
    return in_maps


def kernel(x: np.ndarray, W: np.ndarray, b: np.ndarray) -> np.ndarray:
    x = np.asarray(x, dtype=np.float32)
    W = np.ascontiguousarray(np.asarray(W, dtype=np.float32))
    b = np.asarray(b, dtype=np.float32)
    with_bias = bool(np.any(b))

    nc = _get_nc(with_bias)
    in_maps = _build_in_maps(x, W, b, with_bias)
    res = run_bass_kernel_spmd(nc, in_maps, core_ids=list(range(N_CORES)))
    y = np.concatenate(
        [res.results[c]["yT"].astype(np.float32).T for c in range(N_CORES)],
        axis=0)
    return np.ascontiguousarray(y.reshape(B, S, O))


if __name__ == "__main__":
    rng = np.random.default_rng(0)
    x = rng.standard_normal((B, S, D), dtype=np.float32)
    W = (rng.standard_normal((O, D)) * 0.03).astype(np.float32)
    b = np.zeros((O,), dtype=np.float32)
    y = kernel(x, W, b)
    print("kernel ran, y shape", y.shape, "mean|y|", np.abs(y).mean())
